# revision 18
# baseline (speedup 1.0000x reference)
"""CuPyLinear (sparse CSR y = x @ W.T) Trainium2 kernel, v4.

Problem shapes (hardcoded per spec):
  x       [512, 2048] f32
  data    [262144]    f32   (2048 rows x 128 nnz/row, uniform)
  indices [262144]    i32   (sorted per row, duplicates sum)
  indptr  [2049]      i32   (= arange*128, uniform -> unused on device)
  out y   [512, 2048] f32

v4 design (vs v2's pure row-shard + fp16 PE + full gpsimd densify):
  2D shard: 2 token shards x 4 row shards (cores = (t, rho); each core
  owns 256 tokens x 512 output rows). Math in fp8 e4m3 DoubleRow
  matmuls with first-order error compensation:
      y = xh@Wh + xh@Wl + xl@Wh      (hi/lo fp8 pairs, err ~1.4e-3)
  One DoubleRow instruction contracts two 128-deep k-planes at 0.5
  cycles/output-row, so the 3-term scheme (24 instr per 128-row tile)
  runs well under the fp16 cycle count.

  Weight tiles are PLANE-MAJOR ([p, plane(lo/hi), ct, r]) so every
  matmul lhsT is contiguous in r (walrus Ldweights rejects strided
  innermost APs).  Delivery is hybrid:
   - low ct blocks [0, cd): densified on host, DMA'd into the tile;
   - high ct blocks [cd, 16): compact int16 units DMA'd and placed by
     TWO gpsimd local_scatters per row tile (lo plane, hi plane) into
     disjoint halves of the tile.  A scatter unit is an int16-aligned
     row-PAIR slot (byte0 = even row, byte1 = odd row), so both
     scatters share one idx stream.
  Host prep is weight repacking only (dedupe-sum duplicates, fp8
  quantize, pack the device image); all x-dependent math runs on
  device.  x ships as packed (hi8, lo8) fp8 pairs = 2B/elem (same
  bytes as fp16).
"""

import os
import sys

sys.path.insert(0, "/opt/trn_rl_repo")

import json as _json
from contextlib import ExitStack

import ml_dtypes
import numpy as np

import concourse.bass as bass
import concourse.tile as tile
from concourse import bacc, mybir
from concourse.bass_utils import run_bass_kernel_spmd

P = 128
OUT = 2048
IN = 2048
N = 512
J = 128
NCORES = 8
TSH = 2                   # token shards
RSH = 4                   # row shards
NPT = N // TSH            # 256 tokens per core
ROWS = OUT // RSH         # 512 rows per core
RT = ROWS // P            # 4 row tiles
CT = IN // P              # 16 contraction blocks
E4 = ml_dtypes.float8_e4m3

_CONF = _json.loads(os.environ.get("KCONF", "{}")) if "KCONF" in os.environ else {}
# per-row-tile count of scattered (high) ct blocks; the low 16-c0 ct
# blocks arrive dense via DMA.  Even (main-pass ct pairs share a tile).
C0S = tuple(_CONF.get("c0s", [8, 8, 8, 4]))
assert all(c % 2 == 0 for c in C0S)
CDS = tuple(CT - c for c in C0S)             # dense ct count per rt
WARMUP = _CONF.get("warmup", 30)
# DMA stream order: m=meta, a/b = x halves, w<rt> = dense W
DMA_ORDER = _CONF.get(
    "dma_order", ["m", "a", "b", "w0", "w1", "w2", "w3"]
)
# PE emission order of per-rt dense (d<rt>) / scattered (s<rt>) blocks
MM_ORDER = _CONF.get(
    "mm_order", ["s0", "s1", "d0", "d1", "s2", "s3", "d2", "d3"]
)
# engine for each rt's y copy: 0 = Act(scalar), 1 = DVE(vector); the
# DMA goes through Act for eng 0, SP for eng 1 (DVE can't start DMAs)
Y_ENG = tuple(_CONF.get("y_eng", [0, 1, 0, 1]))
# row tiles whose y DMA is pre-armed as a SWDGE writeback and fired by
# a gpsimd trigger after the PSUM copy (cuts the ~1.4us HWDGE/DGE issue
# latency off the tail); must be in completion order
TRIG_RTS = tuple(_CONF.get("trig_rts", []))

F32 = mybir.dt.float32
FP16 = mybir.dt.float16
FP8 = mybir.dt.float8e4
I16 = mybir.dt.int16

DW_RT = [2 * cd * 64 for cd in CDS]          # dense int16 slots per rt
DW_OFF = np.concatenate([[0], np.cumsum(DW_RT)]).astype(int)
DW = int(DW_OFF[-1])


def build_program(jp):
    nc = bacc.Bacc("TRN2", target_bir_lowering=False, debug=False)

    xt_d = nc.dram_tensor("xt", [P, CT // 2, 2, 2, P], I16, kind="ExternalInput").ap()
    meta_d = nc.dram_tensor("meta", [P, RT, 3 * jp], I16, kind="ExternalInput").ap()
    wd_d = nc.dram_tensor("wd", [P, DW], I16, kind="ExternalInput").ap()
    yt_d = nc.dram_tensor("yt", [RT, P, NPT], FP16, kind="ExternalOutput").ap()

    with tile.TileContext(nc) as tc, ExitStack() as ctx:
        const = ctx.enter_context(tc.tile_pool(name="const", bufs=1))
        mpool = ctx.enter_context(tc.tile_pool(name="meta", bufs=1))
        xpool = ctx.enter_context(tc.tile_pool(name="x", bufs=1))
        wpool = ctx.enter_context(tc.tile_pool(name="w", bufs=1))
        psum_w = ctx.enter_context(tc.tile_pool(name="psum_w", bufs=2, space="PSUM"))
        psum_y = ctx.enter_context(tc.tile_pool(name="psum_y", bufs=1, space="PSUM"))
        ypool = ctx.enter_context(tc.tile_pool(name="y", bufs=2))

        # ---- input DMAs (SP queue; emitted order == transfer order) ----
        mt = mpool.tile([P, RT, 3 * jp], I16, name="mt", tag="mt")
        xa = xpool.tile([P, CT // 4, 2, 2, P], I16, name="xa", tag="xa")
        xb = xpool.tile([P, CT // 4, 2, 2, P], I16, name="xb", tag="xb")
        wds = {
            rt: wpool.tile([P, 2, CDS[rt], 64], I16, name=f"wd{rt}", tag=f"wd{rt}")
            for rt in range(RT) if CDS[rt]
        }
        for tok in DMA_ORDER:
            if tok == "m":
                nc.sync.dma_start(mt[:], meta_d[:, :, :])
            elif tok == "a":
                nc.sync.dma_start(xa[:], xt_d[:, 0 : CT // 4, :, :, :])
            elif tok == "b":
                nc.sync.dma_start(xb[:], xt_d[:, CT // 4 : CT // 2, :, :, :])
            elif tok[0] == "w":
                rt = int(tok[1])
                if rt in wds:
                    nc.sync.dma_start(
                        wds[rt][:], wd_d[:, int(DW_OFF[rt]) : int(DW_OFF[rt + 1])]
                    )
            else:
                raise AssertionError(tok)

        # ---- on-device identity + PE p-state ramp + Act table warm ----
        ii = const.tile([P, P], I16)
        nc.gpsimd.iota(ii[:], [[1, P]], channel_multiplier=-1)
        ident = const.tile([P, P], FP16)
        nc.vector.tensor_scalar(
            ident[:], ii[:], 0.0, None, op0=mybir.AluOpType.is_equal
        )
        actwarm = const.tile([P, 2], F32)
        nc.vector.memset(actwarm[:, 0:1], 0.0)
        nc.scalar.copy(actwarm[:, 1:2], actwarm[:, 0:1])

        for _ in range(WARMUP):
            warm = psum_w.tile([P, P], FP16, space="PSUM", tag="warm")
            nc.tensor.transpose(warm[:], ident[:], ident[:])

        # ---- y staging tiles + pre-armed SWDGE writebacks (idle window) ----
        ysbs = {
            rt: ypool.tile([P, NPT], FP16, name=f"ysb{rt}", tag=f"ysb{rt}")
            for rt in range(RT)
        }
        if TRIG_RTS:
            ydma_sem = nc.alloc_semaphore("ydma")
            yzero = const.tile([P, 1], mybir.dt.int32, name="yzero")
            nc.vector.memset(yzero[:], 0)
            for rt in TRIG_RTS:
                ya = yt_d[rt]
                out4 = bass.AP(
                    ya.tensor, ya.offset,
                    [[NPT * P, 1], [NPT, P], [NPT, 1], [1, NPT]],
                )
                yb = ysbs[rt][:]
                in4 = bass.AP(
                    yb.tensor, yb.offset,
                    [list(yb.ap[0]), [NPT, 1], [NPT, 1], [1, NPT]],
                )
                nc.gpsimd.kv_writeback(
                    out4, in4, yzero[:], prepare_only=True, sem=ydma_sem
                )

        # ---- densify scattered (high) ct range: lo + hi plane scatters ----
        wss = {}
        for rt in range(RT):
            c0 = C0S[rt]
            if c0:
                ws = wpool.tile([P, 2, c0, 64], I16, name=f"ws{rt}", tag=f"ws{rt}")
                idx_ap = mt[:, rt, 2 * jp : 3 * jp]
                nc.gpsimd.local_scatter(
                    ws[:, 0, :, :], mt[:, rt, 0:jp], idx_ap,
                    channels=P, num_elems=c0 * 64, num_idxs=jp,
                )
                nc.gpsimd.local_scatter(
                    ws[:, 1, :, :], mt[:, rt, jp : 2 * jp], idx_ap,
                    channels=P, num_elems=c0 * 64, num_idxs=jp,
                )
                wss[rt] = ws

        # ---- matmuls: per rt, 3-term fp8 DoubleRow per ct pair ----
        def xap(kind, ct):
            xt8 = (xa if ct < 8 else xb).bitcast(FP8)
            pr = (ct // 2) % 4
            if kind == "main":
                return xt8[:, pr, 0, :, :]
            return xt8[:, pr, :, ct % 2, :]

        def wap(rt, kind, ct):
            cd = CDS[rt]
            if ct < cd:
                w8 = wds[rt].bitcast(FP8)        # [P, 2, cd, 128]
                c = ct
            else:
                w8 = wss[rt].bitcast(FP8)        # [P, 2, c0, 128]
                c = ct - cd
            if kind == "main":
                return w8[:, 1, c : c + 2, :]    # (Wh[ct], Wh[ct+1])
            return w8[:, :, c, :]                # (Wl[ct], Wh[ct])

        yps = [
            psum_y.tile([P, NPT], F32, space="PSUM", tag=f"yp{rt}", name=f"yp{rt}")
            for rt in range(RT)
        ]
        n_emitted = [0] * RT
        n_total = [3 * (CT // 2)] * RT
        for blk in MM_ORDER:
            rt = int(blk[1])
            cd = CDS[rt]
            if blk[0] == "d":
                pairs = [2 * t for t in range(cd // 2)]
            else:
                pairs = [2 * t for t in range(cd // 2, CT // 2)]
            for ct in pairs:
                for kind, c in (("main", ct), ("cross", ct), ("cross", ct + 1)):
                    nc.tensor.matmul(
                        yps[rt][:],
                        wap(rt, kind, c),
                        xap(kind, c),
                        start=(n_emitted[rt] == 0),
                        stop=(n_emitted[rt] == n_total[rt] - 1),
                        perf_mode=mybir.MatmulPerfMode.DoubleRow,
                    )
                    n_emitted[rt] += 1
            if n_emitted[rt] == n_total[rt]:
                ysb = ysbs[rt]
                if Y_ENG[rt] == 0:
                    nc.scalar.copy(ysb[:], yps[rt][:])
                else:
                    nc.vector.tensor_copy(ysb[:], yps[rt][:])
                if rt in TRIG_RTS:
                    # the preps deferred their ysb reads to the trigger, so
                    # Tile orders the trigger after all TRIG copies
                    if rt == TRIG_RTS[-1]:
                        nc.gpsimd.trigger_dma(None)
                elif Y_ENG[rt] == 0:
                    nc.scalar.dma_start(yt_d[rt], ysb[:])
                else:
                    nc.sync.dma_start(yt_d[rt], ysb[:])

    nc.compile()
    return nc


# ---------------------------------------------------------------------------
# Host-side plan (static structure from `indices`) + per-call packing
# ---------------------------------------------------------------------------

_PLAN = None


def _build_plan(indices):
    cols = np.asarray(indices).reshape(OUT, J).astype(np.int64)
    rows = np.repeat(np.arange(OUT, dtype=np.int64), J)
    keys = rows * IN + cols.ravel()
    uq, inv = np.unique(keys, return_inverse=True)
    urow = uq // IN
    ucol = uq % IN
    U = len(uq)

    rho = urow // ROWS
    rt = (urow % ROWS) // P
    rloc = urow % P
    p = ucol % P
    ct = ucol // P
    cd = np.asarray(CDS, np.int64)[rt]
    scat = ct >= cd

    # ---- scattered: row-pair units per (rho, rt, p), sorted by target ----
    tgt = (ct - cd) * 64 + rloc // 2             # int16 slot within plane
    shift = (rloc % 2) * 8                       # byte within slot
    si = np.nonzero(scat)[0]
    ukey = ((rho[si] * RT + rt[si]) * P + p[si]) * (CT * 64) + tgt[si]
    order = np.argsort(ukey, kind="stable")
    si = si[order]
    ukey = ukey[order]
    newu = np.concatenate([[True], ukey[1:] != ukey[:-1]])
    unit_id = np.cumsum(newu) - 1                # entry -> unit
    n_units = int(unit_id[-1]) + 1 if len(unit_id) else 0
    # per-unit attributes (from its first entry)
    fi = np.nonzero(newu)[0]
    u_rho = rho[si[fi]]
    u_rt = rt[si[fi]]
    u_p = p[si[fi]]
    u_tgt = tgt[si[fi]]
    grp = (u_rho * RT + u_rt) * P + u_p
    gfirst = np.concatenate([[True], grp[1:] != grp[:-1]])
    gstart = np.zeros(RSH * RT * P, np.int64)
    gstart[grp[gfirst]] = np.nonzero(gfirst)[0]
    slot = np.arange(n_units) - gstart[grp]
    counts = np.bincount(grp, minlength=RSH * RT * P)
    jp = int(counts.max())
    jp += jp % 2
    # meta layout per rho: [P, RT, 3*jp] = lo vals | hi vals | idx
    u_mbase = (u_p * RT + u_rt) * (3 * jp) + slot
    idx_static = []
    for r in range(RSH):
        m = np.zeros((P, RT, 3 * jp), np.int16)
        m[:, :, 2 * jp :] = -1
        idx_static.append(m)
    for r in range(RSH):
        k = u_rho == r
        idx_static[r].reshape(-1)[u_mbase[k] + 2 * jp] = u_tgt[k].astype(np.int16)
    # entry-level fill info: (rho, meta pos of its unit, shift, uq index)
    e_rho = rho[si]
    e_pos = u_mbase[unit_id]
    e_shift = shift[si]
    scat_fill = [
        (e_pos[e_rho == r], e_shift[e_rho == r], si[e_rho == r])
        for r in range(RSH)
    ]

    # ---- dense: int16 positions + byte shift into wd [P, DW] per rho ----
    di = np.nonzero(~scat)[0]
    doff = np.asarray(DW_OFF[:-1], np.int64)[rt[di]]
    cdd = cd[di]
    base = p[di] * DW + doff
    slot_lo = base + ct[di] * 64 + rloc[di] // 2            # plane 0
    slot_hi = slot_lo + cdd * 64                            # plane 1
    dshift = (rloc[di] % 2) * 8
    dsel = rho[di]
    dense_fill = [
        (slot_lo[dsel == r], slot_hi[dsel == r], dshift[dsel == r], di[dsel == r])
        for r in range(RSH)
    ]

    return dict(
        inv=inv, U=U, jp=jp, idx_static=idx_static,
        scat_fill=scat_fill, dense_fill=dense_fill,
    )


def _get_plan(indices):
    global _PLAN
    if _PLAN is None:
        _PLAN = _build_plan(indices)
    return _PLAN


def _quant_pair(vals):
    """f32 -> (lo8, hi8) e4m3 byte arrays."""
    hi = vals.astype(E4)
    lo = (vals - hi.astype(np.float32)).astype(E4)
    return lo.view(np.uint8), hi.view(np.uint8)


def make_in_maps(x, data, indices):
    x = np.asarray(x, dtype=np.float32)
    data = np.asarray(data, dtype=np.float64).ravel()
    pl = _get_plan(indices)
    jp = pl["jp"]

    vals = np.bincount(pl["inv"], weights=data, minlength=pl["U"]).astype(np.float32)
    lo8, hi8 = _quant_pair(vals)

    metas, wdd = [], []
    for r in range(RSH):
        m = pl["idx_static"][r].copy().reshape(-1).view(np.uint16)
        pos, shf, ui = pl["scat_fill"][r]
        np.bitwise_or.at(m, pos, lo8[ui].astype(np.uint16) << shf)
        np.bitwise_or.at(m, pos + jp, hi8[ui].astype(np.uint16) << shf)
        metas.append(m.view(np.int16).reshape(P, RT * 3 * jp))
        w = np.zeros(P * DW, np.uint16)
        plo, phi, shf, ui = pl["dense_fill"][r]
        np.bitwise_or.at(w, plo, lo8[ui].astype(np.uint16) << shf)
        np.bitwise_or.at(w, phi, hi8[ui].astype(np.uint16) << shf)
        wdd.append(w.view(np.int16).reshape(P, DW))

    xh = x.astype(E4)
    xl = (x - xh.astype(np.float32)).astype(E4)
    xts = []
    for t in range(TSH):
        n0 = t * NPT
        arr = np.empty((P, CT // 2, 2, 2, NPT), np.uint8)
        for plidx, src in ((0, xh), (1, xl)):
            s = src[n0 : n0 + NPT].view(np.uint8)       # [NPT, IN]
            s = s.reshape(NPT, CT // 2, 2, P).transpose(3, 1, 2, 0)
            arr[:, :, plidx, :, :] = s
        xts.append(np.ascontiguousarray(arr).view(np.int16))

    in_maps = []
    for core in range(NCORES):
        t, r = core % TSH, core // TSH
        in_maps.append({"xt": xts[t], "meta": metas[r], "wd": wdd[r]})
    return in_maps


_PROGRAM = None
_NEFF_CACHE_DIR = os.path.expanduser("~/.cache/bass_neff")


def _install_neff_disk_cache():
    import hashlib

    import concourse.bass2jax as b2j

    if getattr(b2j.compile_bir_kernel, "_disk_cached", False):
        return
    orig = b2j.compile_bir_kernel

    def cached(bir_json, tmpdir, neff_name="file.neff"):
        canon = bir_json.replace(os.path.abspath(__file__).encode(), b"@KERNEL@")
        key = hashlib.sha256(canon).hexdigest()[:32]
        path = os.path.join(_NEFF_CACHE_DIR, f"{key}.neff")
        out = os.path.join(tmpdir, neff_name)
        if os.path.exists(path):
            import shutil

            shutil.copy(path, out)
            return out
        neff_file = orig(bir_json, tmpdir, neff_name=neff_name)
        try:
            os.makedirs(_NEFF_CACHE_DIR, exist_ok=True)
            tmp = path + ".tmp"
            import shutil

            shutil.copy(neff_file, tmp)
            os.replace(tmp, path)
        except OSError:
            pass
        return neff_file

    cached._disk_cached = True
    b2j.compile_bir_kernel = cached


def _get_program(indices=None):
    global _PROGRAM
    if _PROGRAM is None:
        assert indices is not None, "first _get_program call needs indices"
        _install_neff_disk_cache()
        _PROGRAM = build_program(_get_plan(indices)["jp"])
    return _PROGRAM


def kernel(x, data, indices, indptr):
    nc = _get_program(indices)
    in_maps = make_in_maps(x, data, indices)
    res = run_bass_kernel_spmd(nc, in_maps, core_ids=list(range(NCORES)))
    y = np.empty((N, OUT), np.float32)
    for core in range(NCORES):
        t, r = core % TSH, core // TSH
        yt = np.asarray(res.results[core]["yt"]).astype(np.float32)  # [RT, P, NPT]
        blk = yt.transpose(2, 0, 1).reshape(NPT, ROWS)
        y[t * NPT : (t + 1) * NPT, r * ROWS : (r + 1) * ROWS] = blk
    return np.ascontiguousarray(y)
